# revision 16
# baseline (speedup 1.0000x reference)
"""Multi-head causal attention (B=2, S=2048, d_model=1024, H=16) on 8 Trainium2
NeuronCores.

Sharding: core c -> batch b = c // 4, head group g = c % 4 (heads 4g..4g+3).
Data-parallel over the batch, tensor-parallel over heads: each core computes
QKV projections for its 4 heads (column-sliced Wqkv), causal attention for
those heads, and a partial output projection (row-sliced Wo). The host sums
the 4 partial outputs per batch and adds the output bias.

Device dataflow (per core), all matmuls in fp32r (FP22 single-pass):
  xT [1024, 2048] (host-pre-transposed x[b].T) and W slices live in SBUF.
  qT/kT [per head-pair 128, 2048] = W.T @ x.T via PE (col-partition layout),
  v [2048, 4 heads, 64+1] natural layout with a ones column (row-sum trick).
  Per head: scoresT[j, i] = kT.T @ qT (K=64), causally trimmed; Exp on
  ScalarE (scale=1/8 folded in); diagonal-block triangular mask via DVE;
  AV accumulated over key tiles into PSUM [65, 512] per query chunk (row 64
  accumulates the softmax denominator l). l is copied out, reciprocated
  ([16,128] via a DRAM bounce for partition parallelism) and broadcast back
  through DRAM; values are normalized in SBUF and pair-stacked; the output
  projection contracts K=128 over each head pair into out [2048, 1024].
"""

import sys

sys.path.insert(0, "/opt/trn_rl_repo")

import numpy as np

import concourse.bass as bass
import concourse.mybir as mybir
import concourse.tile as tile
from concourse.bass_utils import run_bass_kernel_spmd

F32 = mybir.dt.float32
F32R = mybir.dt.float32r
BF16 = mybir.dt.bfloat16

B, S, D = 2, 2048, 1024
H_TOT = 16
HD = 64
H_PER_CORE = 4
N_CORES = 8
SCALE = 1.0 / np.sqrt(HD)

ST = S // 128   # 16 sequence tiles of 128
NCH = S // 512  # 4 query chunks of 512


def _split_multi_waits(nc):
    """This container's walrus rejects >1 sem wait per instruction. Move
    extra waits onto fresh single-wait NOPs on the same engine, inserted
    immediately before the instruction (same-engine streams are in-order,
    so semantics are unchanged)."""
    n = 0
    for func in nc.m.functions:
        for bb in func.blocks:
            i = 0
            while i < len(bb.instructions):
                ins = bb.instructions[i]
                si = ins.sync_info
                if si is not None and si.on_wait and len(si.on_wait) > 1:
                    waits = list(si.on_wait)
                    si.on_wait = [waits[-1]]
                    eng = nc.engines[ins.engine]
                    nops = []
                    for w in waits[:-1]:
                        ni = eng.nop(nofuse=True, hint="wait_split").ins
                        if ni.sync_info is None:
                            ni.sync_info = mybir.SyncInfo(on_wait=[w], on_update=[])
                        else:
                            ni.sync_info.on_wait = [w]
                        nops.append(ni)
                    for ni in nops:
                        for f2 in nc.m.functions:
                            for bb2 in f2.blocks:
                                if ni in bb2.instructions:
                                    bb2.instructions.remove(ni)
                    for k, ni in enumerate(nops):
                        bb.instructions.insert(i + k, ni)
                    i += len(nops)
                    n += len(nops)
                i += 1
    return n


def _dram_row_bcast(handle, offset_elems, width, parts):
    """AP that broadcasts a DRAM row of `width` elems across `parts` partitions."""
    return bass.AP(tensor=handle, offset=offset_elems, ap=[[0, parts], [1, width]])


def build_bass(phases=3):
    """phases: 1 = QKV only, 2 = +attention, 3 = full (+projection)."""
    nc = bass.Bass()

    xT = nc.dram_tensor("xT", [D, S], F32R, kind="ExternalInput")
    w = nc.dram_tensor("w", [D, 768], F32R, kind="ExternalInput")
    bias_qk = nc.dram_tensor("bias_qk", [128, 4], F32, kind="ExternalInput")
    bias_v = nc.dram_tensor("bias_v", [256], F32, kind="ExternalInput")
    wo = nc.dram_tensor("wo", [256, D], F32R, kind="ExternalInput")
    tri = nc.dram_tensor("tri", [128, 128], F32R, kind="ExternalInput")
    out = nc.dram_tensor("out", [S, D], BF16, kind="ExternalOutput")

    # Per-queue DMA bandwidth is low in this environment (~18-35 GB/s);
    # round-robin bulk transfers across all three DMA-capable queues
    # (SP-HWDGE, ACT-HWDGE, Pool-SWDGE).
    _dma_engines = [nc.sync, nc.scalar, nc.gpsimd]
    _dma_i = [0]

    def dma_rr(out_ap, in_ap):
        e = _dma_engines[_dma_i[0] % len(_dma_engines)]
        _dma_i[0] += 1
        return e.dma_start(out_ap, in_ap)

    # ACT's queue is only safe while ScalarE is idle (load phase): DMA
    # trigger instructions carry sem waits that would stall the exp stream.
    _dma2_engines = [nc.sync, nc.gpsimd]
    _dma2_i = [0]

    def dma_rr2(out_ap, in_ap):
        e = _dma2_engines[_dma2_i[0] % len(_dma2_engines)]
        _dma2_i[0] += 1
        return e.dma_start(out_ap, in_ap)

    with tile.TileContext(nc) as tc:
        with (
            tc.tile_pool(name="consts", bufs=1) as consts,
            tc.tile_pool(name="qkT_p", bufs=1) as qkT_p,
            tc.tile_pool(name="v_p", bufs=1) as v_p,
            tc.tile_pool(name="values_p", bufs=2) as values_p,
        ):
            # ---- constants ----
            bias_qk_sb = consts.tile([128, 4], F32)
            vbias_bc = consts.tile([128, 256], F32)
            wo_sb = [consts.tile([128, D], F32R, name=f"wo{i}") for i in range(2)]
            tri_sb = consts.tile([128, 128], F32R)
            ones_hi = consts.tile([128, 64], F32R)
            nc.vector.memset(ones_hi[64:65, :].bitcast(F32), 1.0)

            # persistent activation tensors
            qkT = [qkT_p.tile([128, S], F32R, name=f"qkT{mt}") for mt in range(4)]
            v_sb = [v_p.tile([128, H_PER_CORE, 65], F32R, name=f"v{st}") for st in range(ST)]
            values = [
                values_p.tile([128, S], F32R, name=f"vals{hp}", tag="vals")
                for hp in range(2)
            ]

            # ---- phase 1: load x/W, QKV projections ----
            with (
                tc.tile_pool(name="xw_p", bufs=1) as xw_p,
                tc.tile_pool(name="qkv_ps", bufs=3, space="PSUM") as qkv_ps,
                tc.tile_pool(name="vps", bufs=3, space="PSUM") as vps,
            ):
                xt = [xw_p.tile([128, S], F32R, name=f"xt{k}") for k in range(8)]
                wt = [xw_p.tile([128, 768], F32R, name=f"wt{k}") for k in range(8)]
                for k in range(8):
                    dma_rr(xt[k][:], xT[128 * k : 128 * (k + 1), :])
                    dma_rr(wt[k][:], w[128 * k : 128 * (k + 1), :])
                # constants ride the queues behind the critical x/W stream
                nc.sync.dma_start(bias_qk_sb[:], bias_qk[:])
                nc.scalar.dma_start(vbias_bc[:], _dram_row_bcast(bias_v, 0, 256, 128))
                nc.gpsimd.dma_start(tri_sb[:], tri[:])
                for i in range(2):
                    dma_rr(wo_sb[i][:], wo[128 * i : 128 * (i + 1), :])

                # qT/kT: psum[col 128, s 512] accumulated over 8 K-tiles.
                # mt 0/1 = q heads 01/23, mt 2/3 = k heads 01/23.
                # Emit q01/k01 first, then v, then q23/k23, so heads 0-1
                # attention (ACT-bound) can start as early as possible.
                def emit_qk(mt):
                    for ch in range(NCH):
                        pq = qkv_ps.tile([128, 512], F32, name="pq")
                        for k in range(8):
                            nc.tensor.matmul(
                                pq[:],
                                wt[k][:, 128 * mt : 128 * (mt + 1)],
                                xt[k][:, 512 * ch : 512 * (ch + 1)],
                                start=(k == 0),
                                stop=(k == 7),
                            )
                        nc.vector.tensor_scalar(
                            qkT[mt][:, 512 * ch : 512 * (ch + 1)],
                            pq[:],
                            bias_qk_sb[:, mt : mt + 1],
                            None,
                            mybir.AluOpType.add,
                        )

                emit_qk(0)
                emit_qk(2)
                # v natural: psum[s 128, vcol 256] over 8 K-tiles; bias via
                # broadcast TT-add during copy-out; ones column via memset.
                for st in range(ST):
                    pv = vps.tile([128, 256], F32, name="pv")
                    for k in range(8):
                        nc.tensor.matmul(
                            pv[:],
                            xt[k][:, 128 * st : 128 * (st + 1)],
                            wt[k][:, 512:768],
                            start=(k == 0),
                            stop=(k == 7),
                        )
                    nc.vector.memset(v_sb[st][:, :, 64:65].bitcast(F32), 1.0)
                    nc.vector.tensor_tensor(
                        v_sb[st][:, :, 0:64],
                        pv[:].rearrange("p (h d) -> p h d", h=H_PER_CORE),
                        vbias_bc[:].rearrange("p (h d) -> p h d", h=H_PER_CORE).bitcast(F32),
                        mybir.AluOpType.add,
                    )
                emit_qk(1)
                emit_qk(3)

            # ---- phase 2: attention per head ----
            with (
                tc.tile_pool(name="sT_ps", bufs=2, space="PSUM") as sT_ps,
                tc.tile_pool(name="av_ps", bufs=4, space="PSUM") as av_ps,
                tc.tile_pool(name="pT_p", bufs=2) as pT_p,
                tc.tile_pool(name="lone_p", bufs=2) as lone_p,
                tc.tile_pool(name="l16_p", bufs=2) as l16_p,
                tc.tile_pool(name="lr_p", bufs=2) as lr_p,
                tc.tile_pool(name="vtmp_p", bufs=4) as vtmp_p,
            ):
                for h in range(H_PER_CORE if phases >= 2 else 0):
                    hp, hr = h // 2, 64 * (h % 2)
                    q_t = qkT[hp]
                    k_t = qkT[2 + hp]
                    av = [
                        av_ps.tile([128, 512], F32, name=f"av{h}_{ci}", tag="av")
                        for ci in range(NCH)
                    ]
                    l_one = lone_p.tile([128, S], F32, name="l_one")
                    vtmps = [None] * NCH

                    for jt in range(ST):
                        j0 = 128 * jt
                        pT = pT_p.tile([128, S], F32R, name="pT")
                        # process i in [j0, S) in 1024-wide pieces (2 psum banks)
                        for half in range(j0 // 1024, 2):
                            hstart = max(j0, 1024 * half)
                            hend = 1024 * (half + 1)
                            sT = sT_ps.tile([128, 1024], F32, name="sT")
                            c0 = hstart
                            while c0 < hend:
                                c1 = min(hend, (c0 // 512 + 1) * 512)
                                nc.tensor.matmul(
                                    sT[:, c0 - 1024 * half : c1 - 1024 * half],
                                    k_t[hr : hr + 64, j0 : j0 + 128],
                                    q_t[hr : hr + 64, c0:c1],
                                    start=True,
                                    stop=True,
                                )
                                c0 = c1
                            nc.scalar.activation(
                                pT[:, hstart - j0 : hend - j0],
                                sT[:, hstart - 1024 * half : 1024],
                                mybir.ActivationFunctionType.Exp,
                                scale=float(SCALE),
                            )
                            if half == j0 // 1024:
                                # diagonal block: zero the i < j half
                                nc.vector.tensor_tensor(
                                    pT[:, 0:128],
                                    pT[:, 0:128],
                                    tri_sb[:],
                                    mybir.AluOpType.mult,
                                )
                            # AV for the chunks covered by this half
                            for ci in range(2 * half, 2 * half + 2):
                                if 512 * (ci + 1) <= j0:
                                    continue
                                g0 = max(512 * ci, j0)
                                g1 = 512 * (ci + 1)
                                nc.tensor.matmul(
                                    av[ci][0:65, g0 - 512 * ci : 512],
                                    v_sb[jt][:, h, :],
                                    pT[:, g0 - j0 : g1 - j0],
                                    start=(jt == 0),
                                    stop=(jt == 4 * ci + 3),
                                )
                                if jt == 4 * ci + 3:
                                    # chunk ci is final: drain it now so the
                                    # PSUM bank frees early and the tail work
                                    # overlaps the remaining j-tiles.
                                    nc.vector.tensor_copy(
                                        l_one[64:65, 512 * ci : 512 * (ci + 1)],
                                        av[ci][64:65, :],
                                    )
                                    if h % 2 == 0:
                                        nc.vector.tensor_copy(
                                            values[hp][0:64, 512 * ci : 512 * (ci + 1)],
                                            av[ci][0:64, :],
                                        )
                                    else:
                                        vtmps[ci] = vtmp_p.tile(
                                            [64, 512], F32R, name="vtmp"
                                        )
                                        nc.vector.tensor_copy(
                                            vtmps[ci][:], av[ci][0:64, :]
                                        )

                    # ---- softmax denominator: SBUF-only recip + PE broadcast ----
                    l16 = l16_p.tile([16, 128], F32, name="l16")
                    nc.sync.dma_start(l16[:], l_one[64:65, :])
                    nc.vector.reciprocal(l16[:], l16[:])
                    lr = lr_p.tile([128, S], F32R, name="lr")
                    nc.sync.dma_start(lr[64:65, :].bitcast(F32), l16[:])
                    # broadcast 1/l across 64 partitions via K=1 matmuls
                    bct = []
                    for bi in range(2):
                        bt = sT_ps.tile([128, 1024], F32, name=f"bct{bi}", tag="sT")
                        for half2 in range(2):
                            nc.tensor.matmul(
                                bt[0:64, 512 * half2 : 512 * (half2 + 1)],
                                ones_hi[64:65, :],
                                lr[64:65, 1024 * bi + 512 * half2 : 1024 * bi + 512 * (half2 + 1)],
                                start=True,
                                stop=True,
                            )
                        bct.append(bt)

                    # ---- values: normalize (copies already done per-chunk) ----
                    if h % 2 == 0:
                        for bi in range(2):
                            nc.vector.tensor_tensor(
                                values[hp][0:64, 1024 * bi : 1024 * (bi + 1)],
                                values[hp][0:64, 1024 * bi : 1024 * (bi + 1)],
                                bct[bi][0:64, :].bitcast(F32R),
                                mybir.AluOpType.mult,
                            )
                    else:
                        for ci in range(NCH):
                            vtmp = vtmps[ci]
                            nc.vector.tensor_tensor(
                                vtmp[:],
                                vtmp[:],
                                bct[ci // 2][0:64, 512 * (ci % 2) : 512 * (ci % 2 + 1)].bitcast(F32R),
                                mybir.AluOpType.mult,
                            )
                            dma_rr2(
                                values[hp][64:128, 512 * ci : 512 * (ci + 1)], vtmp[:]
                            )

            # ---- phase 3: output projection ----
            with (
                tc.tile_pool(name="proj_ps", bufs=4, space="PSUM") as proj_ps,
                tc.tile_pool(name="out_p", bufs=4) as out_p,
            ):
                if phases < 3:
                    # truncated build: keep an output DMA so the tensor has a
                    # writer, but skip the projection math.
                    out_p_jnk = tc.tile_pool(name="out_jnk", bufs=2).__enter__()
                    for st in range(ST):
                        src = v_sb[st] if phases >= 1 else None
                        jnk = out_p_jnk.tile([128, 260], BF16, name="jnk")
                        nc.vector.tensor_copy(jnk[:], src[:].rearrange("p a b -> p (a b)").bitcast(F32))
                        nc.sync.dma_start(out[128 * st : 128 * (st + 1), 0:260], jnk[:])
                for st in range(ST if phases >= 3 else 0):
                    for nh in range(2):
                        po = proj_ps.tile([128, 512], F32, name="po")
                        for hp in range(2):
                            nc.tensor.matmul(
                                po[:],
                                values[hp][:, 128 * st : 128 * (st + 1)],
                                wo_sb[hp][:, 512 * nh : 512 * (nh + 1)],
                                start=(hp == 0),
                                stop=(hp == 1),
                            )
                        o_sb = out_p.tile([128, 512], BF16, name="o_sb")
                        if (st + nh) % 2 == 0:
                            nc.vector.tensor_copy(o_sb[:], po[:])
                        else:
                            nc.scalar.copy(o_sb[:], po[:])
                        dma_rr2(
                            out[128 * st : 128 * (st + 1), 512 * nh : 512 * (nh + 1)],
                            o_sb[:],
                        )

    _split_multi_waits(nc)
    return nc


_NC_CACHE = None


def _get_nc():
    global _NC_CACHE
    if _NC_CACHE is None:
        _NC_CACHE = build_bass()
    return _NC_CACHE


def make_in_maps(x, mask, Wqkv, bqkv, Wo, bo):
    x = np.asarray(x, dtype=np.float32)
    Wqkv = np.asarray(Wqkv, dtype=np.float32)
    bqkv = np.asarray(bqkv, dtype=np.float32)
    Wo = np.asarray(Wo, dtype=np.float32)

    xT = [np.ascontiguousarray(x[b].T) for b in range(B)]
    tri = (np.arange(128)[None, :] >= np.arange(128)[:, None]).astype(np.float32)

    in_maps = []
    for c in range(N_CORES):
        b, g = c // 4, c % 4
        heads = [4 * g + h for h in range(H_PER_CORE)]
        # Wqkv columns are per-head interleaved: head H -> q cols
        # 192H..192H+64, k cols 192H+64.., v cols 192H+128..
        iq = np.concatenate([np.arange(192 * H, 192 * H + 64) for H in heads])
        ik = np.concatenate([np.arange(192 * H + 64, 192 * H + 128) for H in heads])
        iv = np.concatenate([np.arange(192 * H + 128, 192 * H + 192) for H in heads])
        w_c = np.ascontiguousarray(
            np.concatenate([Wqkv[:, iq], Wqkv[:, ik], Wqkv[:, iv]], axis=1)
        )
        bias_qk = np.stack(
            [bqkv[iq[:128]], bqkv[iq[128:]], bqkv[ik[:128]], bqkv[ik[128:]]],
            axis=1,
        ).astype(np.float32)
        bias_v = np.ascontiguousarray(bqkv[iv])
        wo_c = np.ascontiguousarray(Wo[256 * g : 256 * (g + 1), :])
        in_maps.append(
            {
                "xT": xT[b],
                "w": w_c,
                "bias_qk": bias_qk,
                "bias_v": bias_v,
                "wo": wo_c,
                "tri": tri,
            }
        )
    return in_maps


def bench(x, mask, Wqkv, bqkv, Wo, bo, iters=20):
    """Steady-state timing of the NEFF execution via PJRT with
    device-resident inputs. Returns (best_ns, all_ns)."""
    import time

    import jax
    import jax.numpy as jnp
    from jax.sharding import Mesh, PartitionSpec
    from jax.experimental.shard_map import shard_map
    from concourse import bass2jax
    from concourse.bass2jax import _bass_exec_p, install_neuronx_cc_hook

    install_neuronx_cc_hook()
    nc = _get_nc()
    in_maps = make_in_maps(x, mask, Wqkv, bqkv, Wo, bo)

    partition_name = nc.partition_id_tensor.name if nc.partition_id_tensor else None
    in_names, out_names, out_avals, zero_shapes = [], [], [], []
    for alloc in nc.m.functions[0].allocations:
        if not isinstance(alloc, mybir.MemoryLocationSet):
            continue
        name = alloc.memorylocations[0].name
        if alloc.kind == "ExternalInput":
            if name != partition_name:
                in_names.append(name)
        elif alloc.kind == "ExternalOutput":
            out_names.append(name)
            shape = tuple(alloc.tensor_shape)
            dtype = mybir.dt.np(alloc.dtype)
            out_avals.append(jax.core.ShapedArray(shape, dtype))
            zero_shapes.append((shape, dtype))
    n_params = len(in_names)
    n_outs = len(out_avals)
    all_in_names = list(in_names) + list(out_names)
    if partition_name is not None:
        all_in_names.append(partition_name)

    def _body(*args):
        operands = list(args)
        if partition_name is not None:
            operands.append(bass2jax.partition_id_tensor())
        outs = _bass_exec_p.bind(
            *operands,
            out_avals=tuple(out_avals),
            in_names=tuple(all_in_names),
            out_names=tuple(out_names),
            lowering_input_output_aliases=(),
            sim_require_finite=True,
            sim_require_nnan=True,
            nc=nc,
        )
        return tuple(outs)

    devices = jax.devices()[:N_CORES]
    mesh = Mesh(np.asarray(devices), ("core",))
    donate = tuple(range(n_params, n_params + n_outs))
    sharded = jax.jit(
        shard_map(
            _body,
            mesh=mesh,
            in_specs=(PartitionSpec("core"),) * (n_params + n_outs),
            out_specs=(PartitionSpec("core"),) * n_outs,
            check_rep=False,
        ),
        donate_argnums=donate,
        keep_unused=True,
    )

    concat_in = [
        np.concatenate([np.asarray(in_maps[c][in_names[i]]) for c in range(N_CORES)], axis=0)
        for i in range(n_params)
    ]
    sharding = jax.sharding.NamedSharding(mesh, PartitionSpec("core"))
    dev_in = [jax.device_put(a, sharding) for a in concat_in]

    def make_zeros():
        return [
            jax.device_put(
                np.zeros((N_CORES * s[0], *s[1:]), dt), sharding
            )
            for (s, dt) in zero_shapes
        ]

    # Async python-level chaining: each call donates the previous call's
    # outputs as its output buffers; calls pipeline on the device and we
    # only block at the end. Marginal time over the rep count isolates
    # per-execution device time from fixed RPC/dispatch overhead.
    def timed(reps):
        ts = []
        for _ in range(iters):
            outs = make_zeros()
            for z in outs:
                z.block_until_ready()
            t0 = time.perf_counter()
            for _ in range(reps):
                outs = sharded(*dev_in, *outs)
            for o in outs:
                o.block_until_ready()
            ts.append((time.perf_counter() - t0) * 1e9)
        return ts

    r_lo, r_hi = 1, 129
    t_lo = timed(r_lo)
    t_hi = timed(r_hi)
    best = (min(t_hi) - min(t_lo)) / (r_hi - r_lo)
    med = (sorted(t_hi)[len(t_hi) // 2] - sorted(t_lo)[len(t_lo) // 2]) / (r_hi - r_lo)
    return best, {"lo": t_lo, "hi": t_hi, "marginal_best": best, "marginal_med": med}


def kernel(x, mask, Wqkv, bqkv, Wo, bo, _trace=False):
    nc = _get_nc()
    in_maps = make_in_maps(x, mask, Wqkv, bqkv, Wo, bo)
    res = run_bass_kernel_spmd(nc, in_maps, core_ids=list(range(N_CORES)), trace=_trace)
    partials = [np.asarray(r["out"], dtype=np.float32) for r in res.results]
    bo = np.asarray(bo, dtype=np.float32)
    out = np.empty((B, S, D), dtype=np.float32)
    for b in range(B):
        out[b] = partials[4 * b] + partials[4 * b + 1] + partials[4 * b + 2] + partials[4 * b + 3] + bo
    if _trace:
        return out, res
    return out


# revision 18
# speedup vs baseline: 3.3741x; 3.3741x over previous
"""Multi-head causal attention (B=2, S=2048, d_model=1024, H=16) on 8 Trainium2
NeuronCores.

Sharding: core c -> batch b = c // 4, head group g = c % 4 (heads 4g..4g+3).
Data-parallel over the batch, tensor-parallel over heads: each core computes
QKV projections for its 4 heads (column-sliced Wqkv), causal attention for
those heads, and a partial output projection (row-sliced Wo). The host sums
the 4 partial outputs per batch and adds the output bias.

Device dataflow (per core), all matmuls in fp32r (FP22 single-pass):
  xT [1024, 2048] (host-pre-transposed x[b].T) and W slices live in SBUF.
  qT/kT [per head-pair 128, 2048] = W.T @ x.T via PE (col-partition layout),
  v [2048, 4 heads, 64+1] natural layout with a ones column (row-sum trick).
  Per head: scoresT[j, i] = kT.T @ qT (K=64), causally trimmed; Exp on
  ScalarE (scale=1/8 folded in); diagonal-block triangular mask via DVE;
  AV accumulated over key tiles into PSUM [65, 512] per query chunk (row 64
  accumulates the softmax denominator l). l is copied out, reciprocated
  ([16,128] via a DRAM bounce for partition parallelism) and broadcast back
  through DRAM; values are normalized in SBUF and pair-stacked; the output
  projection contracts K=128 over each head pair into out [2048, 1024].
"""

import sys

sys.path.insert(0, "/opt/trn_rl_repo")

import numpy as np

import concourse.bass as bass
import concourse.mybir as mybir
import concourse.tile as tile
from concourse.bass_utils import run_bass_kernel_spmd

F32 = mybir.dt.float32
F32R = mybir.dt.float32r
BF16 = mybir.dt.bfloat16

B, S, D = 2, 2048, 1024
H_TOT = 16
HD = 64
H_PER_CORE = 4
N_CORES = 8
SCALE = 1.0 / np.sqrt(HD)

ST = S // 128   # 16 sequence tiles of 128
NCH = S // 512  # 4 query chunks of 512


def _split_multi_waits(nc):
    """This container's walrus rejects >1 sem wait per instruction. Move
    extra waits onto fresh single-wait NOPs on the same engine, inserted
    immediately before the instruction (same-engine streams are in-order,
    so semantics are unchanged)."""
    n = 0
    for func in nc.m.functions:
        for bb in func.blocks:
            i = 0
            while i < len(bb.instructions):
                ins = bb.instructions[i]
                si = ins.sync_info
                if si is not None and si.on_wait and len(si.on_wait) > 1:
                    waits = list(si.on_wait)
                    si.on_wait = [waits[-1]]
                    eng = nc.engines[ins.engine]
                    nops = []
                    for w in waits[:-1]:
                        ni = eng.nop(nofuse=True, hint="wait_split").ins
                        if ni.sync_info is None:
                            ni.sync_info = mybir.SyncInfo(on_wait=[w], on_update=[])
                        else:
                            ni.sync_info.on_wait = [w]
                        nops.append(ni)
                    for ni in nops:
                        for f2 in nc.m.functions:
                            for bb2 in f2.blocks:
                                if ni in bb2.instructions:
                                    bb2.instructions.remove(ni)
                    for k, ni in enumerate(nops):
                        bb.instructions.insert(i + k, ni)
                    i += len(nops)
                    n += len(nops)
                i += 1
    return n


def _dram_row_bcast(handle, offset_elems, width, parts):
    """AP that broadcasts a DRAM row of `width` elems across `parts` partitions."""
    return bass.AP(tensor=handle, offset=offset_elems, ap=[[0, parts], [1, width]])


def build_bass(phases=3):
    """phases: 1 = QKV only, 2 = +attention, 3 = full (+projection)."""
    nc = bass.Bass()

    xT = nc.dram_tensor("xT", [D, S], BF16, kind="ExternalInput")
    w = nc.dram_tensor("w", [D, 768], BF16, kind="ExternalInput")
    bias_qk = nc.dram_tensor("bias_qk", [128, 4], F32, kind="ExternalInput")
    bias_v = nc.dram_tensor("bias_v", [256], F32, kind="ExternalInput")
    wo = nc.dram_tensor("wo", [256, D], F32R, kind="ExternalInput")
    tri = nc.dram_tensor("tri", [128, 128], F32R, kind="ExternalInput")
    out = nc.dram_tensor("out", [S, D], BF16, kind="ExternalOutput")

    # Per-queue DMA bandwidth is low in this environment (~18-35 GB/s);
    # round-robin bulk transfers across all three DMA-capable queues
    # (SP-HWDGE, ACT-HWDGE, Pool-SWDGE).
    _dma_engines = [nc.sync, nc.scalar, nc.gpsimd]
    _dma_i = [0]

    def dma_rr(out_ap, in_ap):
        e = _dma_engines[_dma_i[0] % len(_dma_engines)]
        _dma_i[0] += 1
        return e.dma_start(out_ap, in_ap)

    # ACT's queue is only safe while ScalarE is idle (load phase): DMA
    # trigger instructions carry sem waits that would stall the exp stream.
    _dma2_engines = [nc.sync, nc.gpsimd]
    _dma2_i = [0]

    def dma_rr2(out_ap, in_ap):
        e = _dma2_engines[_dma2_i[0] % len(_dma2_engines)]
        _dma2_i[0] += 1
        return e.dma_start(out_ap, in_ap)

    with tile.TileContext(nc) as tc:
        with (
            tc.tile_pool(name="consts", bufs=1) as consts,
            tc.tile_pool(name="qkT_p", bufs=1) as qkT_p,
            tc.tile_pool(name="v_p", bufs=1) as v_p,
            tc.tile_pool(name="values_p", bufs=2) as values_p,
        ):
            # ---- constants ----
            bias_qk_sb = consts.tile([128, 4], F32)
            vbias_bc = consts.tile([128, 256], F32)
            wo_sb = [consts.tile([128, D], F32R, name=f"wo{i}") for i in range(2)]
            tri_sb = consts.tile([128, 128], F32R)
            ones_hi = consts.tile([128, 64], F32R)
            nc.vector.memset(ones_hi[64:65, :].bitcast(F32), 1.0)

            # persistent activation tensors
            qkT = [qkT_p.tile([128, S], F32R, name=f"qkT{mt}") for mt in range(4)]
            v_sb = [v_p.tile([128, H_PER_CORE, 65], F32R, name=f"v{st}") for st in range(ST)]
            values = [
                values_p.tile([128, S], F32R, name=f"vals{hp}", tag="vals")
                for hp in range(2)
            ]

            # ---- phase 1: load x/W, QKV projections ----
            with (
                tc.tile_pool(name="xw_p", bufs=1) as xw_p,
                tc.tile_pool(name="qkv_ps", bufs=3, space="PSUM") as qkv_ps,
                tc.tile_pool(name="vps", bufs=3, space="PSUM") as vps,
            ):
                xt = [xw_p.tile([128, S], BF16, name=f"xt{k}") for k in range(8)]
                wt = [xw_p.tile([128, 768], BF16, name=f"wt{k}") for k in range(8)]
                for k in range(8):
                    dma_rr(xt[k][:], xT[128 * k : 128 * (k + 1), :])
                    dma_rr(wt[k][:], w[128 * k : 128 * (k + 1), :])
                # constants ride the queues behind the critical x/W stream
                nc.sync.dma_start(bias_qk_sb[:], bias_qk[:])
                nc.scalar.dma_start(vbias_bc[:], _dram_row_bcast(bias_v, 0, 256, 128))
                nc.gpsimd.dma_start(tri_sb[:], tri[:])
                for i in range(2):
                    dma_rr(wo_sb[i][:], wo[128 * i : 128 * (i + 1), :])

                # qT/kT: psum[col 128, s 512] accumulated over 8 K-tiles.
                # mt 0/1 = q heads 01/23, mt 2/3 = k heads 01/23.
                # Emit q01/k01 first, then v, then q23/k23, so heads 0-1
                # attention (ACT-bound) can start as early as possible.
                def emit_qk(mt):
                    for ch in range(NCH):
                        pq = qkv_ps.tile([128, 512], F32, name="pq")
                        for k in range(8):
                            nc.tensor.matmul(
                                pq[:],
                                wt[k][:, 128 * mt : 128 * (mt + 1)],
                                xt[k][:, 512 * ch : 512 * (ch + 1)],
                                start=(k == 0),
                                stop=(k == 7),
                            )
                        nc.vector.tensor_scalar(
                            qkT[mt][:, 512 * ch : 512 * (ch + 1)],
                            pq[:],
                            bias_qk_sb[:, mt : mt + 1],
                            None,
                            mybir.AluOpType.add,
                        )

                emit_qk(0)
                emit_qk(2)
                # v natural: psum[s 128, vcol 256] over 8 K-tiles; bias via
                # broadcast TT-add during copy-out; ones column via memset.
                for st in range(ST):
                    pv = vps.tile([128, 256], F32, name="pv")
                    for k in range(8):
                        nc.tensor.matmul(
                            pv[:],
                            xt[k][:, 128 * st : 128 * (st + 1)],
                            wt[k][:, 512:768],
                            start=(k == 0),
                            stop=(k == 7),
                        )
                    nc.vector.memset(v_sb[st][:, :, 64:65].bitcast(F32), 1.0)
                    nc.vector.tensor_tensor(
                        v_sb[st][:, :, 0:64],
                        pv[:].rearrange("p (h d) -> p h d", h=H_PER_CORE),
                        vbias_bc[:].rearrange("p (h d) -> p h d", h=H_PER_CORE).bitcast(F32),
                        mybir.AluOpType.add,
                    )
                emit_qk(1)
                emit_qk(3)

            # ---- phase 2: attention per head ----
            with (
                tc.tile_pool(name="sT_ps", bufs=2, space="PSUM") as sT_ps,
                tc.tile_pool(name="av_ps", bufs=4, space="PSUM") as av_ps,
                tc.tile_pool(name="pT_p", bufs=2) as pT_p,
                tc.tile_pool(name="lone_p", bufs=2) as lone_p,
                tc.tile_pool(name="l16_p", bufs=2) as l16_p,
                tc.tile_pool(name="lr_p", bufs=2) as lr_p,
                tc.tile_pool(name="vtmp_p", bufs=4) as vtmp_p,
            ):
                for h in range(H_PER_CORE if phases >= 2 else 0):
                    hp, hr = h // 2, 64 * (h % 2)
                    q_t = qkT[hp]
                    k_t = qkT[2 + hp]
                    av = [
                        av_ps.tile([128, 512], F32, name=f"av{h}_{ci}", tag="av")
                        for ci in range(NCH)
                    ]
                    l_one = lone_p.tile([128, S], F32, name="l_one")
                    vtmps = [None] * NCH

                    for jt in range(ST):
                        j0 = 128 * jt
                        pT = pT_p.tile([128, S], F32R, name="pT")
                        # process i in [j0, S) in 1024-wide pieces (2 psum banks)
                        for half in range(j0 // 1024, 2):
                            hstart = max(j0, 1024 * half)
                            hend = 1024 * (half + 1)
                            sT = sT_ps.tile([128, 1024], F32, name="sT")
                            c0 = hstart
                            while c0 < hend:
                                c1 = min(hend, (c0 // 512 + 1) * 512)
                                nc.tensor.matmul(
                                    sT[:, c0 - 1024 * half : c1 - 1024 * half],
                                    k_t[hr : hr + 64, j0 : j0 + 128],
                                    q_t[hr : hr + 64, c0:c1],
                                    start=True,
                                    stop=True,
                                )
                                c0 = c1
                            nc.scalar.activation(
                                pT[:, hstart - j0 : hend - j0],
                                sT[:, hstart - 1024 * half : 1024],
                                mybir.ActivationFunctionType.Exp,
                                scale=float(SCALE),
                            )
                            if half == j0 // 1024:
                                # diagonal block: zero the i < j half
                                nc.vector.tensor_tensor(
                                    pT[:, 0:128],
                                    pT[:, 0:128],
                                    tri_sb[:],
                                    mybir.AluOpType.mult,
                                )
                            # AV for the chunks covered by this half
                            for ci in range(2 * half, 2 * half + 2):
                                if 512 * (ci + 1) <= j0:
                                    continue
                                g0 = max(512 * ci, j0)
                                g1 = 512 * (ci + 1)
                                nc.tensor.matmul(
                                    av[ci][0:65, g0 - 512 * ci : 512],
                                    v_sb[jt][:, h, :],
                                    pT[:, g0 - j0 : g1 - j0],
                                    start=(jt == 0),
                                    stop=(jt == 4 * ci + 3),
                                )
                                if jt == 4 * ci + 3:
                                    # chunk ci is final: drain it now so the
                                    # PSUM bank frees early and the tail work
                                    # overlaps the remaining j-tiles.
                                    nc.vector.tensor_copy(
                                        l_one[64:65, 512 * ci : 512 * (ci + 1)],
                                        av[ci][64:65, :],
                                    )
                                    if h % 2 == 0:
                                        nc.vector.tensor_copy(
                                            values[hp][0:64, 512 * ci : 512 * (ci + 1)],
                                            av[ci][0:64, :],
                                        )
                                    else:
                                        vtmps[ci] = vtmp_p.tile(
                                            [64, 512], F32R, name="vtmp"
                                        )
                                        nc.vector.tensor_copy(
                                            vtmps[ci][:], av[ci][0:64, :]
                                        )

                    # ---- softmax denominator: SBUF-only recip + PE broadcast ----
                    l16 = l16_p.tile([16, 128], F32, name="l16")
                    nc.sync.dma_start(l16[:], l_one[64:65, :])
                    nc.vector.reciprocal(l16[:], l16[:])
                    lr = lr_p.tile([128, S], F32R, name="lr")
                    nc.sync.dma_start(lr[64:65, :].bitcast(F32), l16[:])
                    # broadcast 1/l across 64 partitions via K=1 matmuls
                    bct = []
                    for bi in range(2):
                        bt = sT_ps.tile([128, 1024], F32, name=f"bct{bi}", tag="sT")
                        for half2 in range(2):
                            nc.tensor.matmul(
                                bt[0:64, 512 * half2 : 512 * (half2 + 1)],
                                ones_hi[64:65, :],
                                lr[64:65, 1024 * bi + 512 * half2 : 1024 * bi + 512 * (half2 + 1)],
                                start=True,
                                stop=True,
                            )
                        bct.append(bt)

                    # ---- values: normalize (copies already done per-chunk) ----
                    if h % 2 == 0:
                        for bi in range(2):
                            nc.vector.tensor_tensor(
                                values[hp][0:64, 1024 * bi : 1024 * (bi + 1)],
                                values[hp][0:64, 1024 * bi : 1024 * (bi + 1)],
                                bct[bi][0:64, :].bitcast(F32R),
                                mybir.AluOpType.mult,
                            )
                    else:
                        for ci in range(NCH):
                            vtmp = vtmps[ci]
                            nc.vector.tensor_tensor(
                                vtmp[:],
                                vtmp[:],
                                bct[ci // 2][0:64, 512 * (ci % 2) : 512 * (ci % 2 + 1)].bitcast(F32R),
                                mybir.AluOpType.mult,
                            )
                            dma_rr2(
                                values[hp][64:128, 512 * ci : 512 * (ci + 1)], vtmp[:]
                            )

            # ---- phase 3: output projection ----
            with (
                tc.tile_pool(name="proj_ps", bufs=4, space="PSUM") as proj_ps,
                tc.tile_pool(name="out_p", bufs=4) as out_p,
            ):
                if phases < 3:
                    # truncated build: keep an output DMA so the tensor has a
                    # writer, but skip the projection math.
                    out_p_jnk = tc.tile_pool(name="out_jnk", bufs=2).__enter__()
                    for st in range(ST):
                        src = v_sb[st] if phases >= 1 else None
                        jnk = out_p_jnk.tile([128, 260], BF16, name="jnk")
                        nc.vector.tensor_copy(jnk[:], src[:].rearrange("p a b -> p (a b)").bitcast(F32))
                        nc.sync.dma_start(out[128 * st : 128 * (st + 1), 0:260], jnk[:])
                for st in range(ST if phases >= 3 else 0):
                    for nh in range(2):
                        po = proj_ps.tile([128, 512], F32, name="po")
                        for hp in range(2):
                            nc.tensor.matmul(
                                po[:],
                                values[hp][:, 128 * st : 128 * (st + 1)],
                                wo_sb[hp][:, 512 * nh : 512 * (nh + 1)],
                                start=(hp == 0),
                                stop=(hp == 1),
                            )
                        o_sb = out_p.tile([128, 512], BF16, name="o_sb")
                        if (st + nh) % 2 == 0:
                            nc.vector.tensor_copy(o_sb[:], po[:])
                        else:
                            nc.scalar.copy(o_sb[:], po[:])
                        dma_rr2(
                            out[128 * st : 128 * (st + 1), 512 * nh : 512 * (nh + 1)],
                            o_sb[:],
                        )

    _split_multi_waits(nc)
    return nc


_NC_CACHE = None


def _get_nc():
    global _NC_CACHE
    if _NC_CACHE is None:
        _NC_CACHE = build_bass()
    return _NC_CACHE


def make_in_maps(x, mask, Wqkv, bqkv, Wo, bo):
    x = np.asarray(x, dtype=np.float32)
    Wqkv = np.asarray(Wqkv, dtype=np.float32)
    bqkv = np.asarray(bqkv, dtype=np.float32)
    Wo = np.asarray(Wo, dtype=np.float32)

    import ml_dtypes

    xT = [np.ascontiguousarray(x[b].T).astype(ml_dtypes.bfloat16) for b in range(B)]
    tri = (np.arange(128)[None, :] >= np.arange(128)[:, None]).astype(np.float32)

    in_maps = []
    for c in range(N_CORES):
        b, g = c // 4, c % 4
        heads = [4 * g + h for h in range(H_PER_CORE)]
        # Wqkv columns are per-head interleaved: head H -> q cols
        # 192H..192H+64, k cols 192H+64.., v cols 192H+128..
        iq = np.concatenate([np.arange(192 * H, 192 * H + 64) for H in heads])
        ik = np.concatenate([np.arange(192 * H + 64, 192 * H + 128) for H in heads])
        iv = np.concatenate([np.arange(192 * H + 128, 192 * H + 192) for H in heads])
        w_c = np.ascontiguousarray(
            np.concatenate([Wqkv[:, iq], Wqkv[:, ik], Wqkv[:, iv]], axis=1)
        ).astype(ml_dtypes.bfloat16)
        bias_qk = np.stack(
            [bqkv[iq[:128]], bqkv[iq[128:]], bqkv[ik[:128]], bqkv[ik[128:]]],
            axis=1,
        ).astype(np.float32)
        bias_v = np.ascontiguousarray(bqkv[iv])
        wo_c = np.ascontiguousarray(Wo[256 * g : 256 * (g + 1), :])
        in_maps.append(
            {
                "xT": xT[b],
                "w": w_c,
                "bias_qk": bias_qk,
                "bias_v": bias_v,
                "wo": wo_c,
                "tri": tri,
            }
        )
    return in_maps


def bench(x, mask, Wqkv, bqkv, Wo, bo, iters=20):
    """Steady-state timing of the NEFF execution via PJRT with
    device-resident inputs. Returns (best_ns, all_ns)."""
    import time

    import jax
    import jax.numpy as jnp
    from jax.sharding import Mesh, PartitionSpec
    from jax.experimental.shard_map import shard_map
    from concourse import bass2jax
    from concourse.bass2jax import _bass_exec_p, install_neuronx_cc_hook

    install_neuronx_cc_hook()
    nc = _get_nc()
    in_maps = make_in_maps(x, mask, Wqkv, bqkv, Wo, bo)

    partition_name = nc.partition_id_tensor.name if nc.partition_id_tensor else None
    in_names, out_names, out_avals, zero_shapes = [], [], [], []
    for alloc in nc.m.functions[0].allocations:
        if not isinstance(alloc, mybir.MemoryLocationSet):
            continue
        name = alloc.memorylocations[0].name
        if alloc.kind == "ExternalInput":
            if name != partition_name:
                in_names.append(name)
        elif alloc.kind == "ExternalOutput":
            out_names.append(name)
            shape = tuple(alloc.tensor_shape)
            dtype = mybir.dt.np(alloc.dtype)
            out_avals.append(jax.core.ShapedArray(shape, dtype))
            zero_shapes.append((shape, dtype))
    n_params = len(in_names)
    n_outs = len(out_avals)
    all_in_names = list(in_names) + list(out_names)
    if partition_name is not None:
        all_in_names.append(partition_name)

    def _body(*args):
        operands = list(args)
        if partition_name is not None:
            operands.append(bass2jax.partition_id_tensor())
        outs = _bass_exec_p.bind(
            *operands,
            out_avals=tuple(out_avals),
            in_names=tuple(all_in_names),
            out_names=tuple(out_names),
            lowering_input_output_aliases=(),
            sim_require_finite=True,
            sim_require_nnan=True,
            nc=nc,
        )
        return tuple(outs)

    devices = jax.devices()[:N_CORES]
    mesh = Mesh(np.asarray(devices), ("core",))
    donate = tuple(range(n_params, n_params + n_outs))
    sharded = jax.jit(
        shard_map(
            _body,
            mesh=mesh,
            in_specs=(PartitionSpec("core"),) * (n_params + n_outs),
            out_specs=(PartitionSpec("core"),) * n_outs,
            check_rep=False,
        ),
        donate_argnums=donate,
        keep_unused=True,
    )

    concat_in = [
        np.concatenate([np.asarray(in_maps[c][in_names[i]]) for c in range(N_CORES)], axis=0)
        for i in range(n_params)
    ]
    sharding = jax.sharding.NamedSharding(mesh, PartitionSpec("core"))
    dev_in = [jax.device_put(a, sharding) for a in concat_in]

    def make_zeros():
        return [
            jax.device_put(
                np.zeros((N_CORES * s[0], *s[1:]), dt), sharding
            )
            for (s, dt) in zero_shapes
        ]

    # Async python-level chaining: each call donates the previous call's
    # outputs as its output buffers; calls pipeline on the device and we
    # only block at the end. Marginal time over the rep count isolates
    # per-execution device time from fixed RPC/dispatch overhead.
    def timed(reps):
        ts = []
        for _ in range(iters):
            outs = make_zeros()
            for z in outs:
                z.block_until_ready()
            t0 = time.perf_counter()
            for _ in range(reps):
                outs = sharded(*dev_in, *outs)
            for o in outs:
                o.block_until_ready()
            ts.append((time.perf_counter() - t0) * 1e9)
        return ts

    r_lo, r_hi = 1, 65
    t_lo = timed(r_lo)
    t_hi = timed(r_hi)
    best = (min(t_hi) - min(t_lo)) / (r_hi - r_lo)
    med = (sorted(t_hi)[len(t_hi) // 2] - sorted(t_lo)[len(t_lo) // 2]) / (r_hi - r_lo)
    return best, {"lo": t_lo, "hi": t_hi, "marginal_best": best, "marginal_med": med}


def kernel(x, mask, Wqkv, bqkv, Wo, bo, _trace=False):
    nc = _get_nc()
    in_maps = make_in_maps(x, mask, Wqkv, bqkv, Wo, bo)
    res = run_bass_kernel_spmd(nc, in_maps, core_ids=list(range(N_CORES)), trace=_trace)
    partials = [np.asarray(r["out"], dtype=np.float32) for r in res.results]
    bo = np.asarray(bo, dtype=np.float32)
    out = np.empty((B, S, D), dtype=np.float32)
    for b in range(B):
        out[b] = partials[4 * b] + partials[4 * b + 1] + partials[4 * b + 2] + partials[4 * b + 3] + bo
    if _trace:
        return out, res
    return out
